# revision 1
# baseline (speedup 1.0000x reference)
"""Trainium2 Bass kernel for nn_BinarizedConv2d.

Math: activation[d, o] = sum_k weight_noise[d, o, k] * x[d, k]
      out[d, o]        = activation[d, o] > bias_noise[d, o]
with D=128 directions, O=256 out channels, K=2304 reduction length.

Sharding: D is split across 8 NeuronCores (16 directions per core) —
embarrassingly parallel, no collectives.

dtype trick: weight_noise and x are exactly 0/1, which fp8e4 represents
exactly; matmul accumulation is always fp32 in PSUM, and popcounts <= 2304
are exact in fp32, so results are bit-identical to the fp32 reference while
moving 4x fewer bytes from HBM (this kernel is HBM-bandwidth-bound on the
9.4 MB/core weight stream).

Per-core kernel: directions are processed as 4 "quads" mapped onto the four
32-column groups of the PE array (tile_position=(0, 32j)), so 4 matvecs run
concurrently. For each K-tile of 128, direction 4q+j's x column is the
stationary operand, broadcast over 32 PE columns with a step-0 AP (M=32);
the pre-transposed weight K-tile [128, 256] is the moving operand (N=256).
The 18 K-tiles accumulate in partitions 32j..32j+31 of PSUM bank q, so each
quad's epilogue is a single fused VectorE op ((psum + 0) is_gt bias) against
a partition-replicated bias, plus a per-quad uint8 store.

Weights stream as half-quad chunks (1.18 MB, contiguous per partition),
alternating between the two HWDGE rings (SP/ACT); the last half is split in
3 so the PE only trails the stream end by ~3 K-tiles. Every DVE/PE
instruction is structured to need at most ONE semaphore wait (the 64B TPB
instruction structs have a single wait slot): per-quad result tiles avoid
write-after-write waits, and the bias replication (0-step-AP DMA broadcast)
is covered by a DVE probe copy so compares only wait on the PE.
"""

import numpy as np
import ml_dtypes

D = 128          # directions (ES population)
O = 256          # out channels
K = 2304         # flattened reduction length
T = 18           # K tiles of 128
P = 128          # partitions / K-tile size
NCORES = 8
DPC = D // NCORES  # directions per core

FP8 = ml_dtypes.float8_e4m3

_nc_cache = {}


def _emit(tc, res_ap, wT_ap, xT_ap, bias_ap):
    """Emit the per-core program into TileContext tc."""
    import concourse.mybir as mybir

    nc = tc.nc
    fp8 = mybir.dt.float8e4
    f32 = mybir.dt.float32
    u8 = mybir.dt.uint8

    NQ = DPC // 4  # quads of 4 directions, col-tiled across the PE array
    TH = T // 2    # k-tiles per half (W DMA'd in halves for pipelining)

    with (
        tc.tile_pool(name="w", bufs=1) as wp,
        tc.tile_pool(name="small", bufs=1) as sp,
        tc.tile_pool(name="act", bufs=1) as ap_pool,
        tc.tile_pool(name="ps", bufs=1, space="PSUM") as pp,
    ):
        # x first on the SP ring so no W chunk queues ahead of it (every
        # matmul depends on x).
        x_tile = sp.tile([P, DPC * T], fp8)
        nc.sync.dma_start(out=x_tile[:], in_=xT_ap)
        # W arrives as half-quad chunks of [P, TH*4*O] (1.18 MB, contiguous
        # per partition for big SDMA descriptors; th-major so k-tile ranges
        # are contiguous), issued in consume order and alternating between
        # the two HWDGE rings (SP + ACT) so both descriptor queues stream
        # concurrently. The final half is split into 3 pieces so the PE only
        # trails the stream end by ~3 k-tiles.
        NPIECE = 3
        PTH = TH // NPIECE
        HW_ = TH * 4 * O     # elements per half
        PW = PTH * 4 * O     # elements per piece
        w_quads = []
        ring = [nc.sync, nc.scalar]
        issue = 0
        for q in range(NQ):
            halves = []
            for h in range(2):
                if q < NQ - 1 or h == 0:
                    wt = wp.tile([P, HW_], fp8, tag=f"wq{q}h{h}")
                    ring[issue % 2].dma_start(
                        out=wt[:], in_=wT_ap[q][:, h * HW_ : (h + 1) * HW_]
                    )
                    issue += 1
                    halves.append(wt)
                else:
                    pieces = []
                    for pz in range(NPIECE):
                        wt = wp.tile([P, PW], fp8, tag=f"wq{q}h{h}p{pz}")
                        ring[issue % 2].dma_start(
                            out=wt[:],
                            in_=wT_ap[q][:, HW_ + pz * PW : HW_ + (pz + 1) * PW],
                        )
                        issue += 1
                        pieces.append(wt)
                    halves.append(pieces)
            w_quads.append(halves)

        # bias arrives as 4 DRAM rows (row j = directions 4q+j over quads q),
        # each DMA'd with a 0-step partition AP so partition 32j+r holds
        # direction 4q+j's bias for all r. Issued after the W chunks so the
        # stream-critical weight data is not queued behind the 128 KB of
        # replicated bias writes; ring FIFO still lands bias well before the
        # first quad's epilogue. A DVE probe copy then absorbs the DMA wait
        # so the fused compares only ever wait on the PE semaphore (the TPB
        # 64B instruction structs have a single sync-wait slot).
        bias_rep = sp.tile([P, NQ * O], f32)
        for j in range(4):
            nc.scalar.dma_start(
                out=bias_rep[32 * j : 32 * (j + 1), :],
                in_=bias_ap[j : j + 1, :].broadcast_to((32, NQ * O)),
            )
        probe_tile = sp.tile([1, 4], f32)
        nc.vector.tensor_copy(out=probe_tile[:], in_=bias_rep[0:1, 0:4])

        # One PSUM tile spanning all 8 banks; quad q accumulates in bank q's
        # first 256 columns. Direction j of a quad accumulates in partition
        # rows 32j..32j+31 via PE column-group tiling, so the 4 matvecs run
        # concurrently in the array (independent 32-col groups) and the quad
        # epilogue is full-width on DVE. skip_group_check: the per-(q,j)
        # accumulation groups are disjoint (partition x bank), but the group
        # tracker models PSUM flat and can't represent partition-ranged
        # groups; actual has_written accumulate semantics are per element.
        ps_all = pp.tile([P, 8 * 2 * O], f32)
        for q in range(NQ):
            win = slice(q * 2 * O, q * 2 * O + O)
            for t in range(T):
                h, th = divmod(t, TH)
                src = w_quads[q][h]
                if isinstance(src, list):
                    src = src[th // PTH]
                    th = th % PTH
                for j in range(4):
                    d = q * 4 + j
                    # lhsT is x broadcast over 32 columns (step-0 AP): all 32
                    # rows of PE column-group j compute the same matvec, so
                    # the activation fills partitions 32j..32j+31.
                    nc.tensor.matmul(
                        ps_all[32 * j : 32 * (j + 1), win],
                        x_tile[:, d * T + t : d * T + t + 1].broadcast_to((P, 32)),
                        src[:, (th * 4 + j) * O : (th * 4 + j + 1) * O],
                        start=(t == 0),
                        stop=(t == T - 1),
                        tile_position=(0, 32 * j),
                        skip_group_check=True,
                    )
            sl = slice(q * O, (q + 1) * O)
            # Fused epilogue: res = (ps + 0.0) is_gt bias, one DVE op per
            # quad, reading PSUM directly. Per-quad res tiles: no WAW between
            # quads, so each op's only semaphore wait is the PE one.
            res_q = ap_pool.tile([P, O], u8, tag=f"res{q}")
            nc.vector.scalar_tensor_tensor(
                out=res_q[:],
                in0=ps_all[:, win],
                scalar=0.0,
                in1=bias_rep[:, sl],
                op0=mybir.AluOpType.add,
                op1=mybir.AluOpType.is_gt,
            )
            # Per-quad result store: quads 0..2 fly out while later quads
            # still compute; only quad 3's small store is on the tail.
            nc.scalar.dma_start(out=res_ap[:, sl], in_=res_q[0:P:32, :])


def _build():
    """Build the per-core Bass program (same NEFF on all 8 cores)."""
    import concourse.bacc as bacc
    import concourse.mybir as mybir
    from concourse.tile import TileContext

    # Bacc (not raw Bass): its compile() runs move_matmul_waits_to_ldweights,
    # which splits 2-wait matmuls into LDW-wait + MM-wait (the 64B TPB
    # instruction structs have a single sync-wait slot).
    nc = bacc.Bacc("TRN2", debug=False, enable_asserts=False)

    fp8 = mybir.dt.float8e4
    f32 = mybir.dt.float32
    u8 = mybir.dt.uint8

    # wT[q, p, ((h*9+th)*4 + j)*O + o] = weight_noise[d0+4q+j, o, (h*9+th)*128+p]
    # (pre-transposed host side; one region per quad, h/th-major)
    wT = nc.dram_tensor("wT", [DPC // 4, P, T * 4 * O], fp8, kind="ExternalInput")
    # xT[p, d*T + t] = x[d0+d, t*128+p]
    xT = nc.dram_tensor("xT", [P, DPC * T], fp8, kind="ExternalInput")
    # bias[j, q*O + o] = bias_noise[d0+4q+j, o]
    bias = nc.dram_tensor("bias", [4, (DPC // 4) * O], f32, kind="ExternalInput")
    # res[j, q*O + o] = out[d0+4q+j, o]
    res = nc.dram_tensor("res", [4, (DPC // 4) * O], u8, kind="ExternalOutput")

    with TileContext(nc) as tc:
        _emit(tc, res.ap(), wT.ap(), xT.ap(), bias.ap())
    nc.compile()
    return nc


def prepare_inputs(weight_noise, bias_noise, x):
    """Host-side dtype cast + layout transform + sharding. Exact (0/1 -> fp8)."""
    w8 = np.asarray(weight_noise).astype(FP8)           # [D, O, K]
    # wT[d, p, t, o] = w[d, o, t*128+p]
    wT = np.ascontiguousarray(
        w8.reshape(D, O, T, P).transpose(0, 3, 2, 1)
    ).reshape(D, P, T * O)
    x8 = np.asarray(x).astype(FP8)                      # [D, K]
    xTfull = np.ascontiguousarray(x8.reshape(D, T, P).transpose(2, 0, 1))  # [P, D, T]
    b32 = np.asarray(bias_noise).astype(np.float32)

    in_maps = []
    for c in range(NCORES):
        sl = slice(c * DPC, (c + 1) * DPC)
        # [d, p, t, o] -> [q, p, t, j, o] -> one region per quad (t-major)
        wc = (
            wT[sl]
            .reshape(DPC // 4, 4, P, T, O)
            .transpose(0, 2, 3, 1, 4)
            .reshape(DPC // 4, P, T * 4 * O)
        )
        # bias[j, q*O+o] = bias_noise[d0 + 4q + j, o]
        bc = (
            b32[sl]
            .reshape(DPC // 4, 4, O)
            .transpose(1, 0, 2)
            .reshape(4, (DPC // 4) * O)
        )
        in_maps.append(
            {
                "wT": np.ascontiguousarray(wc),
                "xT": np.ascontiguousarray(xTfull[:, sl, :]).reshape(P, DPC * T),
                "bias": np.ascontiguousarray(bc),
            }
        )
    return in_maps


def run(weight_noise, bias_noise, x, trace=False, **spmd_kwargs):
    """Run on the 8 NeuronCores; returns (bool [D, O] output, BassKernelResults)."""
    from concourse.bass_utils import run_bass_kernel_spmd

    in_maps = prepare_inputs(weight_noise, bias_noise, x)
    if "nc" in _nc_cache:
        nc = _nc_cache["nc"]
    else:
        nc = _nc_cache["nc"] = _build()
    r = run_bass_kernel_spmd(
        nc, in_maps, core_ids=list(range(NCORES)), trace=trace, **spmd_kwargs
    )
    out = np.concatenate(
        [
            r.results[c]["res"]
            .reshape(4, DPC // 4, O)
            .transpose(1, 0, 2)
            .reshape(DPC, O)
            for c in range(NCORES)
        ],
        axis=0,
    )
    return out.astype(bool), r


def kernel(weight_noise, bias_noise, x):
    out, _ = run(weight_noise, bias_noise, x)
    return out



# revision 6
# speedup vs baseline: 1.3918x; 1.3918x over previous
"""Trainium2 Bass kernel for nn_BinarizedConv2d (2-bit-packed weight stream).

Math: activation[d, o] = sum_k weight_noise[d, o, k] * x[d, k]
      out[d, o]        = activation[d, o] > bias_noise[d, o]
with D=128 directions, O=256 out channels, K=2304 reduction length.
Sharding: D split across 8 NeuronCores (16 directions per core), no
collectives.

Key idea: weights and x are 0/1 bits, so adjacent k-pairs are packed host-
side into ONE fp8 byte p = w_even + 2*w_odd (exact float values {0,1,2,3}),
halving the HBM weight stream to 4.72 MB/core (this kernel is HBM-bound).
On-chip, the second operand stream is reconstructed with a single DVE op:
the fp8e4m3 encodings of {0,1,2,3} are {0x00,0x38,0x40,0x44}, so
(enc & 0x40) is the encoding of 2*w_odd exactly. Doing the AND on a
uint16-bitcast view processes 2 bytes/element and qualifies for the DVE
4x_2P perf mode (single-src, SBUF, 16-bit) -> ~4.8us for all 4.7M weights.

The matvec then uses TWO accumulating matmul streams per direction:
  sum_m xe[m]*p[m] + (0.5*xo[m] - xe[m]) * q[m]
    = sum_m xe*we + xo*wo          (exact; all partial products integers)
where xe/xo are the even/odd x bits (host-split layout, device computes the
combined coefficient c2 = 0.5*xo - xe with one DVE op).

PE mapping as in the fp8 baseline: directions in quads on the four 32-col
PE groups (tile_position=(0,32j)), x-coefficient broadcast as stationary,
packed-weight tiles as moving operand, PSUM bank q per quad, fused
(psum+0) is_gt bias epilogue per quad on DVE.

Scheduling fixes vs baseline: bias broadcast goes over SWDGE (gpsimd) so it
never queues ahead of/behind weights on the HWDGE rings; weight chunks are
small at the start (PE starts ~8.5us instead of 19us) and split at the end
(PE trails the stream end closely); all four quads' results land in one
SBUF tile and fly out in a single small DMA instead of four.
"""

import numpy as np
import ml_dtypes

D = 128          # directions (ES population)
O = 256          # out channels
K = 2304         # flattened reduction length
NT = 9           # packed k-tiles of 128 (K/2 = 1152 pairs)
P = 128          # partitions
NCORES = 8
DPC = D // NCORES  # directions per core
NQ = DPC // 4      # quads per core

FP8 = ml_dtypes.float8_e4m3

_nc_cache = {}


def _emit(tc, res_ap, wT_ap, xT_ap, bias_ap):
    """Emit the per-core program into TileContext tc."""
    import concourse.mybir as mybir

    nc = tc.nc
    fp8 = mybir.dt.float8e4
    u16 = mybir.dt.uint16
    f32 = mybir.dt.float32
    u8 = mybir.dt.uint8
    XN = DPC * NT  # 144 coefficient columns per stream

    with (
        tc.tile_pool(name="w", bufs=1) as wp,
        tc.tile_pool(name="small", bufs=1) as sp,
        tc.tile_pool(name="act", bufs=1) as ap_pool,
        tc.tile_pool(name="ps", bufs=1, space="PSUM") as pp,
    ):
        # bias replication via SWDGE (gpsimd): separate descriptor queue, so
        # it neither delays the weight rings nor lands late like the
        # baseline (where bias queued behind 4.8 MB of weights and stalled
        # the epilogue by ~5us). 4 rows -> 32-way partition broadcast.
        bias_rep = sp.tile([P, NQ * O], f32)
        for j in range(4):
            nc.gpsimd.dma_start(
                out=bias_rep[32 * j : 32 * (j + 1), :],
                in_=bias_ap[j : j + 1, :].broadcast_to((32, NQ * O)),
            )

        # x even/odd bit streams first on the SP ring (everything depends on
        # them): xeo[:, :XN] = xe, xeo[:, XN:] = xo.
        xeo = sp.tile([P, 2 * XN], fp8)
        nc.sync.dma_start(out=xeo[:], in_=xT_ap)

        # Packed weight stream p: one region per quad, t-major, chunked so
        # the PE can start early and trails the stream end closely.
        CH = {
            0: [(0, 2), (2, 5), (5, 9)],
            1: [(0, 4), (4, 9)],
            2: [(0, 4), (4, 9)],
            3: [(0, 3), (3, 6), (6, 9)],
        }
        ring = [nc.sync, nc.scalar]
        issue = 0
        p_tiles = []
        chunks = []  # (quad, c0, c1) in emission order for the derive ops
        for q in range(NQ):
            pt = wp.tile([P, NT * 4 * O], fp8, tag=f"p{q}")
            p_tiles.append(pt)
            for (t0, t1) in CH[q]:
                c0, c1 = t0 * 4 * O, t1 * 4 * O
                ring[issue % 2].dma_start(out=pt[:, c0:c1], in_=wT_ap[q][:, c0:c1])
                issue += 1
                chunks.append((q, c0, c1))

        # Derived stream q = p & 0x4040 on a uint16 view = 2*w_odd in fp8,
        # exactly. Single-src SBUF->SBUF uint16 => DVE 4x mode.
        q_tiles = []
        for q in range(NQ):
            q_t = wp.tile([P, NT * 4 * O], fp8, tag=f"q{q}", name=f"q_t{q}")
            q_tiles.append(q_t)

        # Coefficient stream for the derived operand: c2 = 0.5*xo - xe.
        c2 = sp.tile([P, XN], fp8)
        nc.vector.scalar_tensor_tensor(
            out=c2[:],
            in0=xeo[:, XN:],
            scalar=0.5,
            in1=xeo[:, :XN],
            op0=mybir.AluOpType.mult,
            op1=mybir.AluOpType.subtract,
        )

        def derive(qi, c0, c1):
            src = p_tiles[qi][:, c0:c1].bitcast(u16)
            dst = q_tiles[qi][:, c0:c1].bitcast(u16)
            nc.vector.tensor_scalar(
                out=dst, in0=src, scalar1=0x4040, scalar2=None,
                op0=mybir.AluOpType.bitwise_and,
            )

        # Results for all quads in one tile; one small store at the end.
        res_all = ap_pool.tile([P, NQ * O], u8)

        def compare(qi):
            # probe-free: bias lands (SWDGE, ~9us) long before any quad
            # finishes; the compare's waits are the PE sem + bias sem.
            nc.vector.scalar_tensor_tensor(
                out=res_all[:, qi * O : (qi + 1) * O],
                in0=ps_all[:, qi * 2 * O : qi * 2 * O + O],
                scalar=0.0,
                in1=bias_rep[:, qi * O : (qi + 1) * O],
                op0=mybir.AluOpType.add,
                op1=mybir.AluOpType.is_gt,
            )

        ps_all = pp.tile([P, 8 * 2 * O], f32)

        # Derives first (in data-arrival order). Compares are emitted AFTER
        # the matmuls: emission order is program order for the dependency
        # tracker, so a compare emitted before the matmuls would read
        # unwritten PSUM and stall the matmuls on a false WAR hazard.
        for (qi, c0, c1) in chunks:
            derive(qi, c0, c1)

        # Matmuls: per quad, stream p over all 9 tiles, then stream q.
        # Direction j of a quad lives in PE column-group j / PSUM partitions
        # 32j..32j+31 (as in the baseline).
        for q in range(NQ):
            win = slice(q * 2 * O, q * 2 * O + O)
            for s in range(2):
                src_t = p_tiles[q] if s == 0 else q_tiles[q]
                coef = xeo if s == 0 else c2
                for t in range(NT):
                    for j in range(4):
                        d = q * 4 + j
                        nc.tensor.matmul(
                            ps_all[32 * j : 32 * (j + 1), win],
                            coef[:, d * NT + t : d * NT + t + 1].broadcast_to((P, 32)),
                            src_t[:, (t * 4 + j) * O : (t * 4 + j + 1) * O],
                            start=(s == 0 and t == 0),
                            stop=(s == 1 and t == NT - 1),
                            tile_position=(0, 32 * j),
                            skip_group_check=True,
                        )

        for q in range(NQ):
            compare(q)

        # Single result store: rows 0,32,64,96 hold directions j=0..3.
        nc.sync.dma_start(out=res_ap[:, :], in_=res_all[0:P:32, :])


def _build():
    """Build the per-core Bass program (same NEFF on all 8 cores)."""
    import concourse.bacc as bacc
    import concourse.mybir as mybir
    from concourse.tile import TileContext

    nc = bacc.Bacc("TRN2", debug=False, enable_asserts=False)

    fp8 = mybir.dt.float8e4
    f32 = mybir.dt.float32
    u8 = mybir.dt.uint8

    # wT[q, p, (t*4 + j)*O + o] = packed pair stream for direction d0+4q+j,
    # pair index m = t*128 + p, value w[2m] + 2*w[2m+1] in fp8.
    wT = nc.dram_tensor("wT", [NQ, P, NT * 4 * O], fp8, kind="ExternalInput")
    # xT[p, s*144 + d*9 + t] = x[d0+d, 2*(t*128+p) + s] for s in {0=even,1=odd}
    xT = nc.dram_tensor("xT", [P, 2 * DPC * NT], fp8, kind="ExternalInput")
    # bias[j, q*O + o] = bias_noise[d0+4q+j, o]
    bias = nc.dram_tensor("bias", [4, NQ * O], f32, kind="ExternalInput")
    # res[j, q*O + o] = out[d0+4q+j, o]
    res = nc.dram_tensor("res", [4, NQ * O], u8, kind="ExternalOutput")

    with TileContext(nc) as tc:
        _emit(tc, res.ap(), wT.ap(), xT.ap(), bias.ap())
    nc.compile()
    return nc


def prepare_inputs(weight_noise, bias_noise, x):
    """Host-side dtype cast + pair packing + layout transform + sharding.

    All transforms are data-independent (fixed index shuffles and the exact
    0/1 -> fp8 pack); the actual reduction/compare math runs on device.
    """
    w = np.asarray(weight_noise)                      # [D, O, K] 0/1 floats
    wpair = w.reshape(D, O, K // 2, 2)
    pvals = (wpair[..., 0] + 2.0 * wpair[..., 1]).astype(FP8)   # [D, O, 1152]
    # [D, O, NT, P] -> [D, P, NT, O]
    pT = np.ascontiguousarray(pvals.reshape(D, O, NT, P).transpose(0, 3, 2, 1))

    xb = np.asarray(x)
    xe = xb[:, 0::2].astype(FP8).reshape(D, NT, P)    # [D, NT, P]
    xo = xb[:, 1::2].astype(FP8).reshape(D, NT, P)
    xeT = np.ascontiguousarray(xe.transpose(2, 0, 1))  # [P, D, NT]
    xoT = np.ascontiguousarray(xo.transpose(2, 0, 1))

    b32 = np.asarray(bias_noise).astype(np.float32)

    in_maps = []
    for c in range(NCORES):
        sl = slice(c * DPC, (c + 1) * DPC)
        # [d, p, t, o] -> [q, j, p, t, o] -> [q, p, t, j, o]
        wc = (
            pT[sl]
            .reshape(NQ, 4, P, NT, O)
            .transpose(0, 2, 3, 1, 4)
            .reshape(NQ, P, NT * 4 * O)
        )
        xc = np.concatenate(
            [xeT[:, sl, :].reshape(P, DPC * NT), xoT[:, sl, :].reshape(P, DPC * NT)],
            axis=1,
        )
        bc = (
            b32[sl]
            .reshape(NQ, 4, O)
            .transpose(1, 0, 2)
            .reshape(4, NQ * O)
        )
        in_maps.append(
            {
                "wT": np.ascontiguousarray(wc),
                "xT": np.ascontiguousarray(xc),
                "bias": np.ascontiguousarray(bc),
            }
        )
    return in_maps


def run(weight_noise, bias_noise, x, trace=False, **spmd_kwargs):
    """Run on the 8 NeuronCores; returns (bool [D, O] output, results)."""
    from concourse.bass_utils import run_bass_kernel_spmd

    in_maps = prepare_inputs(weight_noise, bias_noise, x)
    if "nc" in _nc_cache:
        nc = _nc_cache["nc"]
    else:
        nc = _nc_cache["nc"] = _build()
    r = run_bass_kernel_spmd(
        nc, in_maps, core_ids=list(range(NCORES)), trace=trace, **spmd_kwargs
    )
    out = np.concatenate(
        [
            r.results[c]["res"]
            .reshape(4, NQ, O)
            .transpose(1, 0, 2)
            .reshape(DPC, O)
            for c in range(NCORES)
        ],
        axis=0,
    )
    return out.astype(bool), r


def kernel(weight_noise, bias_noise, x):
    out, _ = run(weight_noise, bias_noise, x)
    return out
